# revision 45
# baseline (speedup 1.0000x reference)
"""Multi-head self-attention with RoPE, sharded over 8 TRN2 NeuronCores.

Sharding: tensor-parallel over heads (2 heads/core) for QKV projections and
attention; AllToAlls redistribute attention outputs from head-sharded to
sequence-sharded so each core computes 1/8 of the output projection rows.

Device-side layout choices (host pre-stages everything):
- x is passed transposed (xt = x.T) so projection matmuls contract naturally.
- Wq/Wk rows are pair-permuted (evens then odds per head) so RoPE becomes
  rotate-half form; the 1/sqrt(hd) score scale is folded into Wq.
- Scores are computed transposed (S^T = K @ Q^T, keys on partitions) so the
  softmax denominator comes free from an ones-column appended to V, and P^T
  feeds the PV matmul with no on-device transpose of P. The two heads' score
  matmuls have K=64 on partition halves 0-63/64-127, so they auto-row-tile
  (64x128) and run concurrently on the PE array.
- Attention outputs are normalized per chunk BEFORE the AllToAll (approx
  reciprocal + e01-broadcast matmul), so the collective payload is contiguous
  bf16 and the consumer side needs no normalization stage.
- The output redistribution runs in `npass` AllToAll passes: each core's
  output query shard is the union of one sub-shard per pass, so every A2A
  slot carries real data (no zero padding) and each pass's output projection
  overlaps later attention chunks.
- proj+rope of chunk sc+1 is emitted before attention of chunk sc so the
  DVE rope work overlaps the PE/ACT attention work of the previous chunk.
- All matmuls run as bf16 (full PE rate); rope tables/temps are bf16.

Hardcoded problem shape: B=1, S=4096, D=1024, H=16, hd=64, theta=10000.
"""

import math

import numpy as np

import concourse.bass as bass
import concourse.mybir as mybir
import concourse.tile as tile
from concourse import bacc
from concourse.bass_utils import run_bass_kernel_spmd

N_CORES = 8
D_MODEL = 1024
NUM_HEADS = 16
HEAD_DIM = 64
THETA = 10000.0
P = 128  # partitions; also = 2 heads x 64 dims per core
KD = D_MODEL // 128  # 8 contraction tiles for the projections

F32 = mybir.dt.float32
F32R = mybir.dt.float32r
BF16 = mybir.dt.bfloat16
EXP = mybir.ActivationFunctionType.Exp

ADT = BF16  # attention matmul dtype (x, Wqkv, Q/K, V, P)

NPASS = (4, 2, 1, 1)  # chunks per A2A redistribution pass (sums to NCH)
OPRO_DELAY = 2  # chunks to wait after a pass's A2A before its out-proj

# debug toggles for HW bisecting (set via kernel.DBG[...] = True; clear
# _BUILD_CACHE after changing)
DBG = {
    "recip_sbuf": False,   # reciprocal from an SBUF copy instead of PSUM
    "f32_vt": False,       # f32 V-transpose path (baseline behavior)
    "slot_dma": False,     # per-slot simple DMAs instead of batched strided
    "no_flush": False,     # no head-split flush epilogue
    "rb_matmul": False,    # e01-matmul broadcast instead of gpsimd
}

# Schraudolph bf16-exp on DVE for a subset of late-chunk non-diagonal
# blocks (ScalarE saturates there while the DVE is idle). exp(x) ~=
# bitcast_bf16(int16(x * 128/ln2 + B)); ~1.8% rms / 4% max relative error.
DVE_EXP_MINQC = 5   # route blocks of chunks >= this
DVE_EXP_MOD = 3     # ... when ki % mod == 0 (non-diag only)
SCHRAUD_A = 128.0 / math.log(2.0)
SCHRAUD_B = 16248.6


def _pass_spec(nch, npass):
    """Normalize npass (int = even split, tuple = chunks per pass)."""
    if isinstance(npass, int):
        cpp = max(1, nch // npass)
        sizes = [cpp] * npass
    else:
        sizes = list(npass)
    assert sum(sizes) == nch, (sizes, nch)
    starts = [sum(sizes[:p]) for p in range(len(sizes))]
    return sizes, starts


def build(seq: int, p12_reps: int = 1, p3_reps: int = 1, parts: str = "full",
          npass=NPASS):
    """Build the SPMD Bass program for sequence length `seq`.

    p12_reps > 1 wraps phases 1+2 (projections + attention) in an on-device
    For_i loop; p3_reps > 1 unrolls phase 3 (A2A + out-proj) — both exist
    for wall-clock timing above the axon dispatch floor. Defaults give the
    normal single-shot kernel.
    """
    CH = min(512, seq)          # free-dim chunk for matmuls / PSUM banks
    NCH = seq // CH             # number of seq chunks
    KB = seq // 128             # key blocks
    KBC = CH // 128             # key blocks per chunk (4 at CH=512)
    SW = seq // N_CORES         # per-core output seq shard
    P_SIZES, P_STARTS = _pass_spec(NCH, npass)
    NP = len(P_SIZES)
    SWPS = [CH * s // N_CORES for s in P_SIZES]   # per-pass slot widths
    OFFS = [sum(SWPS[:p]) for p in range(NP)]     # out_d column offsets
    PASS_OF = sum(([p] * s for p, s in enumerate(P_SIZES)), [])

    nc = bacc.Bacc("TRN2", num_devices=N_CORES)

    xt = nc.dram_tensor("xt", [D_MODEL, seq], ADT, kind="ExternalInput")
    wq = nc.dram_tensor("wq", [P, D_MODEL], ADT, kind="ExternalInput")
    wk = nc.dram_tensor("wk", [P, D_MODEL], ADT, kind="ExternalInput")
    wv = nc.dram_tensor("wv", [P, D_MODEL], ADT, kind="ExternalInput")
    wo = nc.dram_tensor("wo", [P, KD * D_MODEL], BF16, kind="ExternalInput")
    cstab = nc.dram_tensor("cstab", [P, 2 * seq], BF16, kind="ExternalInput")
    dmaskd = nc.dram_tensor("dmask", [P, KBC * CH], BF16,
                            kind="ExternalInput")
    ident = nc.dram_tensor("ident", [P, 128],
                           F32 if DBG["f32_vt"] else BF16,
                           kind="ExternalInput")
    onesd = nc.dram_tensor("ones", [P, max(KB, 64)], ADT, kind="ExternalInput")
    e01d = nc.dram_tensor("e01", [2, P], F32R, kind="ExternalInput")
    out_d = nc.dram_tensor("out", [D_MODEL, SW], BF16, kind="ExternalOutput")

    with tile.TileContext(nc) as tc:
        with (
            tc.tile_pool(name="const", bufs=1) as cpool,
            tc.tile_pool(name="mats", bufs=1) as mpool,
            tc.tile_pool(name="xt", bufs=2) as xpool,
            tc.tile_pool(name="sc", bufs=2) as spool,
            tc.tile_pool(name="pt", bufs=8) as ptpool,
            tc.tile_pool(name="at", bufs=2) as atpool,
            tc.tile_pool(name="ps", bufs=2, space="PSUM") as pspool,
            tc.tile_pool(name="pss", bufs=2, space="PSUM") as psspool,
            tc.tile_pool(name="dram", bufs=1, space="DRAM") as dpool,
        ):
            # ---- q-proj weights first: chunk 0 depends on them ----
            w_sb = {}
            for name, src in (("q", wq), ("k", wk), ("v", wv)):
                t = cpool.tile([P, D_MODEL], ADT, tag=f"w{name}",
                               name=f"w_{name}")
                w_sb[name] = t
            # k0-tile first so chunk 0's first projection matmul can start
            # as soon as ~160KB (not 1.3MB) has landed
            nc.sync.dma_start(out=w_sb["q"][:, 0:128], in_=wq[:, 0:128])
            nc.sync.dma_start(out=w_sb["q"][:, 128:D_MODEL],
                              in_=wq[:, 128:D_MODEL])
            dmask = idn = ones = e01 = None
            wot = []

            def emit_kv_weights():
                nc.sync.dma_start(out=w_sb["k"][:], in_=wk[:])
                nc.sync.dma_start(out=w_sb["v"][:], in_=wv[:])

            def emit_consts():
                """Light constants — emitted after chunk 0's input DMAs
                so they don't delay the first projection."""
                nonlocal dmask, idn, ones, e01
                dmask = cpool.tile([P, KBC * CH], BF16, tag="dmask")
                nc.sync.dma_start(out=dmask[:], in_=dmaskd[:])
                idn = cpool.tile([P, 128],
                                 F32 if DBG["f32_vt"] else BF16, tag="idn")
                nc.sync.dma_start(out=idn[:], in_=ident[:])
                ones = cpool.tile([P, max(KB, 64)], ADT, tag="ones")
                nc.sync.dma_start(out=ones[:], in_=onesd[:])
                e01 = {}
                for h in (0, 1):
                    t = cpool.tile([1, P], F32R, tag=f"e01{h}")
                    nc.sync.dma_start(out=t[:], in_=e01d[h:h + 1, :])
                    e01[h] = t
                # ones columns of vnat (cols 64 and 129 of each 130-block)
                vv = vnat[:].rearrange("p (k c) -> p k c", c=130)
                oo = ones[:, 0:KB].rearrange("p (k c) -> p k c", c=1)
                nc.vector.tensor_copy(vv[:, :, 64:65], oo)
                nc.vector.tensor_copy(vv[:, :, 129:130], oo)

            def emit_wo():
                """Out-proj weights — deferred past the early xt pressure."""
                t = cpool.tile([P, KD * D_MODEL], BF16, tag="wo", name="wo_t")
                nc.sync.dma_start(out=t[:], in_=wo[:])
                for e in range(KD):
                    wot.append(t[:, bass.ts(e, D_MODEL)])

            # ---- persistent matrices ----
            qT = mpool.tile([P, seq], ADT, tag="qT")  # rows: 2 heads x 64
            kT = mpool.tile([P, seq], ADT, tag="kT")
            vnat = mpool.tile([P, KB * 130], ADT, tag="vnat")

            a2a_in = [dpool.tile([N_CORES, P, SWPS[p]], BF16,
                                 tag=f"a2a_in{p}", name=f"a2a_in_{p}")
                      for p in range(NP)]
            a2a_out = [dpool.tile([N_CORES, P, SWPS[p]], BF16,
                                  tag=f"a2a_out{p}", name=f"a2a_out_{p}")
                       for p in range(NP)]

            def emit_proj_dmas(sc):
                """Issue the input DMAs for chunk sc; returns the tiles.

                All KD k-tiles of x land with ONE DMA (dma_start queue
                processing is ~650ns per instruction, so batching matters)."""
                sl = bass.ts(sc, CH)
                xa = xpool.tile([P, KD * CH], ADT, tag="xts",
                                name=f"xt_{sc}")
                if sc < 2:
                    # ramp: per-k DMAs so the first projection matmul can
                    # start after 128KB instead of 1MB
                    for k in range(KD):
                        nc.sync.dma_start(
                            out=xa[:, bass.ts(k, CH)],
                            in_=xt[128 * k:128 * (k + 1), sl],
                        )
                else:
                    nc.sync.dma_start(
                        out=xa[:].rearrange("p (k c) -> p k c", c=CH),
                        in_=xt.rearrange("(k p) s -> p k s", p=128)[:, :, sl],
                    )
                xts = [xa[:, bass.ts(k, CH)] for k in range(KD)]
                cs_c = spool.tile([P, 2 * CH], BF16, tag="cs", name=f"cs_{sc}")
                nc.sync.dma_start(out=cs_c[:], in_=cstab[:, bass.ts(sc, 2 * CH)])
                return xts, cs_c

            def proj_qk_units(sc, staged):
                """Generator of PE-op units for chunk sc's Q/K projections +
                rope, interleavable between attention blocks. DVE ops are
                emitted inline at their dependency points."""
                sl = bass.ts(sc, CH)
                xts, cs_c = staged
                ct_c = cs_c[:, 0:CH]
                st_c = cs_c[:, CH:2 * CH]
                if parts == "dma":
                    return
                sw_c = {}
                for nm in ("qs", "ks"):
                    sw_c[nm] = spool.tile([P, CH], BF16, tag=nm,
                                          name=f"sw_{sc}_{nm}")
                for name, dst in (("q", qT[:, sl]), ("k", kT[:, sl])):
                    ps = pspool.tile([P, CH], F32, tag="mm",
                                     name=f"proj_{sc}_{name}")
                    for k in range(KD):
                        nc.tensor.matmul(
                            ps[:],
                            w_sb[name][:, bass.ts(k, 128)],
                            xts[k][:],
                            start=(k == 0),
                            stop=(k == KD - 1),
                        )
                        if k % 2 == 1:
                            yield
                    nc.vector.tensor_copy(dst, ps[:])
                    if parts == "proj":
                        continue
                    # rope: mat = mat*cos + swapped*sin (swapped halves via
                    # DVE copies)
                    mat = qT if name == "q" else kT
                    eng = nc.vector
                    swc = sw_c["qs" if name == "q" else "ks"]
                    for h in (0, 1):
                        for half in (0, 1):
                            d0 = 64 * h + 32 * half
                            s0 = 64 * h + 32 * (1 - half)
                            eng.tensor_copy(
                                swc[d0:d0 + 32, :], mat[s0:s0 + 32, sl]
                            )
                    tm = spool.tile([P, CH], BF16, tag=f"tmp_{name}",
                                    name=f"tmp_{sc}_{name}")
                    eng.tensor_mul(tm[:], swc[:], st_c)
                    eng.tensor_mul(mat[:, sl], mat[:, sl], ct_c)
                    eng.tensor_add(mat[:, sl], mat[:, sl], tm[:])

            def proj_v_units(sc, staged):
                """V projection + per-block transpose into vnat for chunk sc
                (needed only by chunk sc's own diagonal PV, so it drains
                later than the Q/K units)."""
                xts, _ = staged
                if parts == "dma":
                    return
                vdt = F32 if DBG["f32_vt"] else BF16
                vt_c = spool.tile([P, CH], vdt, tag="vt", name=f"vt_{sc}")
                ps = pspool.tile([P, CH], F32, tag="mm",
                                 name=f"proj_{sc}_v")
                for k in range(KD):
                    nc.tensor.matmul(
                        ps[:],
                        w_sb["v"][:, bass.ts(k, 128)],
                        xts[k][:],
                        start=(k == 0),
                        stop=(k == KD - 1),
                    )
                    if k % 2 == 1:
                        yield
                nc.vector.tensor_copy(vt_c[:], ps[:])
                if parts in ("proj", "rope"):
                    return
                for j in range(KBC):
                    kb = sc * KBC + j
                    pst = pspool.tile([P, 2 * CH] if not DBG["f32_vt"]
                                      else [P, CH], vdt, tag="mm",
                                      name=f"vtr_{kb}")
                    nc.tensor.transpose(
                        pst[:, 0:128], vt_c[:, bass.ts(j, 128)], idn[:]
                    )
                    # one strided copy fills both head halves (cols 0..63 and
                    # 65..128); the ones columns at 64/129 sit in the stride
                    # gap and stay untouched
                    dstv = vnat[:, 130 * kb:130 * (kb + 1)].rearrange(
                        "p (g c) -> p g c", c=65)
                    srcv = pst[:, 0:128].rearrange("p (g c) -> p g c", c=64)
                    nc.vector.tensor_copy(dstv[:, :, 0:64], srcv)
                    yield

            # Global queue of ((chunk, kind), proj-unit generator) in need
            # order: kind 0 = Q/K+rope (needed when attn(chunk) starts),
            # kind 1 = V+transpose (needed at attn(chunk)'s diagonal).
            # Attention blocks pump one PE unit per block so projection work
            # spreads evenly instead of clumping at chunk boundaries.
            proj_queue = []

            def pump(n=1):
                while n > 0 and proj_queue:
                    try:
                        next(proj_queue[0][1])
                        n -= 1
                    except StopIteration:
                        proj_queue.pop(0)

            def ensure_proj_done(key):
                while proj_queue and proj_queue[0][0] <= key:
                    for _ in proj_queue[0][1]:
                        pass
                    proj_queue.pop(0)

            def drain_all():
                ensure_proj_done((NCH, 1))

            def emit_attn_chunk(qc, prev_epi=None, flush=False):
                """Attention for query chunk qc (needs proj chunks 0..qc).

                Per key block: S^T for both heads lands in one [128, 2*CH]
                PSUM tile ([0:CH]=h0, [CH:2CH]=h1) so a single wide exp
                covers both heads. Diagonal-band blocks are processed LAST
                (their masking runs on GPSIMD and gets latency-hidden behind
                the non-diagonal tail of the PV accumulation). Chunk qc+1's
                projection PE units are pumped between blocks to fill
                TensorE slack while ScalarE streams the exps.
                """
                kbmax = (qc + 1) * KBC
                psu = {}
                for h in (0, 1):
                    psu[h] = pspool.tile([65, CH], F32, tag="u",
                                         name=f"psu_{qc}_{h}")
                # old blocks first: they only need this chunk's roped Q, so
                # attention can start before rope-K of this chunk finishes;
                # the diagonal band comes last (its GPSIMD masks still hide
                # behind the deferred-PV pipeline)
                kb_order = (list(range(0, kbmax - KBC))
                            + list(range(kbmax - KBC, kbmax)))

                def emit_pv(kb, ki, pt, qoff, heads=(0, 1)):
                    for h in heads:
                        nc.tensor.matmul(
                            psu[h][:, qoff:CH],
                            vnat[:, 130 * kb + 65 * h:
                                 130 * kb + 65 * (h + 1)],
                            pt[:, CH * h + qoff:CH * (h + 1)],
                            start=(ki == 0),
                            stop=(ki == kbmax - 1),
                        )

                pending = []  # PVs deferred two blocks: (kb, ki, pt, qoff)
                for ki, kb in enumerate(kb_order):
                    j = kb - (kbmax - KBC)  # diag index if >= 0
                    if j == 0:
                        # own-chunk V/vnat needed from the first diag PV on
                        ensure_proj_done((qc, 1))
                    # diag block j only contributes to queries >= 128j; skip
                    # the fully-masked left region in scores, exp, and PV
                    qoff = 128 * j if j > 0 else 0
                    pss = psspool.tile([P, 2 * CH], F32, tag="s",
                                       name=f"sc_{qc}_{kb}")
                    for h in (0, 1):
                        nc.tensor.matmul(
                            pss[:, CH * h + qoff:CH * (h + 1)],
                            kT[64 * h:64 * (h + 1), bass.ts(kb, 128)],
                            qT[64 * h:64 * (h + 1),
                               CH * qc + qoff:CH * (qc + 1)],
                            start=True,
                            stop=True,
                        )
                    if parts == "attn_sc":
                        pump()
                        continue
                    pt = ptpool.tile([P, 2 * CH], ADT, tag="pt",
                                     name=f"pt_{qc}_{kb}")
                    use_dve = (j < 0 and qc >= DVE_EXP_MINQC
                               and ki % DVE_EXP_MOD == 0)
                    if use_dve:
                        nc.vector.tensor_scalar(
                            out=pt[:].bitcast(mybir.dt.int16),
                            in0=pss[:],
                            scalar1=SCHRAUD_A, scalar2=SCHRAUD_B,
                            op0=mybir.AluOpType.mult,
                            op1=mybir.AluOpType.add,
                        )
                    elif qoff == 0:
                        nc.scalar.activation(pt[:], pss[:], EXP)
                    else:
                        for h in (0, 1):
                            nc.scalar.activation(
                                pt[:, CH * h + qoff:CH * (h + 1)],
                                pss[:, CH * h + qoff:CH * (h + 1)], EXP)
                    if j >= 0:
                        # triangle mask on the [128j, 128j+128) query range
                        for h in (0, 1):
                            nc.gpsimd.tensor_mul(
                                pt[:, CH * h + 128 * j:
                                   CH * h + 128 * (j + 1)],
                                pt[:, CH * h + 128 * j:
                                   CH * h + 128 * (j + 1)],
                                dmask[:, CH * j + 128 * j:
                                      CH * j + 128 * (j + 1)],
                            )
                    if parts == "attn_s":
                        pump()
                        if ki == 1 and prev_epi is not None:
                            prev_epi()
                            prev_epi = None
                        continue
                    if len(pending) >= 3:
                        emit_pv(*pending.pop(0))
                    pending.append((kb, ki, pt, qoff))
                    pump()
                    if ki == 1 and prev_epi is not None:
                        # previous chunk's epilogue, deferred past this
                        # chunk's first scores so ScalarE never starves on
                        # the sums->reciprocal->broadcast chain
                        prev_epi()
                        prev_epi = None
                # --- epilogue pieces: normalize U by the softmax sums (row
                # 64 of each psu) and ship bf16 sub-shard slices into this
                # pass's A2A input ---
                from concourse.dve_ops import (
                    RECIP_APPROX_FAST_CONSTS as _RC,
                    RECIPROCAL_APPROX_FAST as _RA,
                )
                epi_state = {}

                def epi_head(h, on_act=False):
                    uu = epi_state.get("uu")
                    if uu is None:
                        uu = spool.tile([P, CH], F32, tag="uu",
                                        name=f"uu_{qc}")
                        epi_state["uu"] = uu
                    # at flush points ScalarE is between chunks and idle, so
                    # h0's PSUM evacuations run there, parallel to h1's on
                    # DVE; deferred epilogues keep everything on DVE (ACT is
                    # mid-exp-stream there). The custom-DVE reciprocal can't
                    # read PSUM on HW, so the sums row bounces through SBUF.
                    rs = spool.tile([1, CH], F32R, tag=f"rs{h}",
                                    name=f"rs_{qc}_{h}")
                    sm = spool.tile([1, CH], F32, tag=f"sm{h}",
                                    name=f"sm_{qc}_{h}")
                    if on_act:
                        nc.scalar.copy(uu[64 * h:64 * (h + 1), :],
                                       psu[h][0:64, :])
                        nc.scalar.copy(sm[:], psu[h][64:65, :])
                    else:
                        nc.vector.tensor_copy(uu[64 * h:64 * (h + 1), :],
                                              psu[h][0:64, :])
                        nc.vector.tensor_copy(sm[:], psu[h][64:65, :])
                    nc.vector._custom_dve(
                        _RA, out=rs[:], in0=sm[:],
                        s0=_RC["s0"], s1=_RC["s1"], imm2=_RC["imm2"],
                    )
                    epi_state[f"rs{h}"] = rs

                def epi_finish():
                    uu = epi_state["uu"]
                    if DBG["rb_matmul"]:
                        rb = pspool.tile([P, CH], F32, tag="mm",
                                         name=f"rb_{qc}")
                        for h in (0, 1):
                            nc.tensor.matmul(rb[:], e01[h][:],
                                             epi_state[f"rs{h}"][:],
                                             start=(h == 0), stop=(h == 1))
                        rbv = rb
                    else:
                        # per-head reciprocal rows broadcast on GPSIMD
                        # (keeps the PE out of the epilogue chain)
                        rbs = spool.tile([P, CH], F32R, tag="rbs",
                                         name=f"rbs_{qc}")
                        for h in (0, 1):
                            nc.gpsimd.partition_broadcast(
                                rbs[64 * h:64 * (h + 1), :],
                                epi_state[f"rs{h}"][:], channels=64)
                        rbv = rbs
                    ut = ptpool.tile([P, CH], BF16, tag="ut", name=f"ut_{qc}")
                    nc.vector.tensor_mul(ut[:], uu[:], rbv[:])
                    pp = PASS_OF[qc]
                    m = qc - P_STARTS[pp]
                    spc = CH // SWPS[pp]
                    if DBG["slot_dma"]:
                        for i in range(spc):
                            nc.sync.dma_start(
                                out=a2a_in[pp][spc * m + i],
                                in_=ut[:, bass.ts(i, SWPS[pp])],
                            )
                    else:
                        nc.sync.dma_start(
                            out=a2a_in[pp][spc * m:spc * (m + 1)]
                            .rearrange("i p c -> p i c"),
                            in_=ut[:].rearrange("p (i c) -> p i c",
                                                c=SWPS[pp]),
                        )

                if prev_epi is not None:
                    prev_epi()
                if parts not in ("attn_s", "attn_sc"):
                    if flush:
                        # pass-end chunk: drain per head so the h0 epilogue
                        # DVE work overlaps the h1 PV drain, shortening the
                        # serial chain in front of this pass's collective
                        for p in pending:
                            emit_pv(*p, heads=(0,))
                        epi_head(0)
                        for p in pending:
                            emit_pv(*p, heads=(1,))
                    else:
                        for p in pending:
                            emit_pv(*p)
                if parts in ("attn_s", "attn_sc", "attn_pv"):
                    return None
                if flush:
                    epi_head(1)
                    epi_finish()
                    return None

                def epilogue():
                    epi_head(0)
                    epi_head(1)
                    epi_finish()

                return epilogue

            def emit_cc(ab_in, ab_out):
                nc.gpsimd.collective_compute(
                    "AllToAll",
                    mybir.AluOpType.bypass,
                    replica_groups=[list(range(N_CORES))],
                    ins=[ab_in.opt()],
                    outs=[ab_out.opt()],
                )

            def emit_pass_outproj(pp):
                """At-loads + out-projection + store for pass pp. Emitted at
                least OPRO_DELAY chunks after pass pp's A2A fired so the PE
                queue never stalls waiting on the collective."""
                swp = SWPS[pp]
                ata = atpool.tile([P, N_CORES * swp], BF16,
                                  tag=f"at{pp}", name=f"at_{pp}")
                if DBG["slot_dma"]:
                    for i in range(N_CORES):
                        nc.sync.dma_start(
                            out=ata[:, bass.ts(i, swp)],
                            in_=a2a_out[pp][i],
                        )
                else:
                    nc.sync.dma_start(
                        out=ata[:].rearrange("p (i c) -> p i c", c=swp),
                        in_=a2a_out[pp][:].rearrange("i p c -> p i c"),
                    )
                ota = atpool.tile([P, KD * swp], BF16,
                                  tag=f"ot{pp}", name=f"ot_{pp}")
                for e in range(KD):
                    # alternate PSUM pools and evacuation engines so the
                    # 8 accumulation groups pipeline 4-deep instead of
                    # round-tripping through one pool + one engine
                    pool = pspool if e % 2 == 0 else psspool
                    tag = "mm" if e % 2 == 0 else "s"
                    pso = pool.tile([P, CH], F32, tag=tag,
                                    name=f"pso_{pp}_{e}")
                    for i in range(N_CORES):
                        nc.tensor.matmul(
                            pso[:, 0:swp],
                            wot[e][:, bass.ts(i, 128)],
                            ata[:, swp * i:swp * (i + 1)],
                            start=(i == 0),
                            stop=(i == N_CORES - 1),
                        )
                    if e % 2 == 0:
                        nc.vector.tensor_copy(
                            ota[:, swp * e:swp * (e + 1)], pso[:, 0:swp])
                    else:
                        nc.scalar.copy(
                            ota[:, swp * e:swp * (e + 1)], pso[:, 0:swp])
                if DBG["slot_dma"]:
                    for e in range(KD):
                        nc.sync.dma_start(
                            out=out_d[bass.ts(e, 128),
                                      OFFS[pp]:OFFS[pp] + swp],
                            in_=ota[:, bass.ts(e, swp)],
                        )
                else:
                    nc.sync.dma_start(
                        out=out_d.rearrange("(e p) c -> p e c", p=128)
                        [:, :, OFFS[pp]:OFFS[pp] + swp],
                        in_=ota[:].rearrange("p (e c) -> p e c", c=swp),
                    )

            def emit_p12(fired, staged0=None):
                # proj is staged 2 chunks ahead of attention; its PE units
                # go through the global proj_queue, pumped one per attention
                # block so projection work spreads evenly over the phase and
                # the ScalarE exp stream stays dense.
                attn_on = parts not in ("dma", "proj", "rope")

                def stage(sc):
                    staged = emit_proj_dmas(sc)
                    proj_queue.append(((sc, 0), proj_qk_units(sc, staged)))
                    proj_queue.append(((sc, 1), proj_v_units(sc, staged)))

                if staged0 is not None:
                    proj_queue.append(((0, 0), proj_qk_units(0, staged0)))
                    proj_queue.append(((0, 1), proj_v_units(0, staged0)))
                else:
                    stage(0)
                if NCH > 1:
                    stage(1)
                epi = None
                pending_op = []
                for sc in range(NCH):
                    if sc == 1:
                        emit_wo()
                    if sc + 2 < NCH:
                        stage(sc + 2)
                    ensure_proj_done((sc, 0))
                    if attn_on:
                        pp = PASS_OF[sc]
                        pass_end = sc == P_STARTS[pp] + P_SIZES[pp] - 1
                        epi = emit_attn_chunk(
                            sc, epi,
                            flush=pass_end and not DBG["no_flush"])
                        if pass_end and pp < NP - 1 and p12_reps == 1:
                            if epi is not None:
                                epi()
                                epi = None
                            emit_cc(a2a_in[pp], a2a_out[pp])
                            fired.append(pp)
                            # out-proj for fired passes lands after the LAST
                            # chunk's attention: mid-phase PE is saturated,
                            # but the final collective leaves it idle
                            pending_op.append((pp, NCH - 1))
                        while pending_op and pending_op[0][1] <= sc:
                            emit_pass_outproj(pending_op.pop(0)[0])
                if epi is not None:
                    epi()
                drain_all()
                pending_op_tail.extend(pp for pp, _ in pending_op)

            def emit_p3(fired):
                # leftover outprojs first (their data already landed; they
                # fill the PE while the final collective flies)
                todo = [pp for pp in range(NP) if pp not in fired]
                for pp in todo:
                    emit_cc(a2a_in[pp], a2a_out[pp])
                    while pending_op_tail:
                        emit_pass_outproj(pending_op_tail.pop(0))
                for pp in todo:
                    emit_pass_outproj(pp)

            fired = []
            pending_op_tail = []
            if p12_reps == 1:
                staged0 = emit_proj_dmas(0)
                emit_kv_weights()
                emit_consts()
                emit_p12(fired, staged0)
            else:
                emit_kv_weights()
                emit_consts()
                with tc.For_i(0, p12_reps, 1):
                    emit_p12(fired)
            for r3 in range(p3_reps):
                emit_p3(fired if r3 == 0 else list(fired))

    nc.finalize()
    return nc


def prepare_in_maps(in_features, token_positions, Wq, Wk, Wv, Wo, seq):
    """Host-side staging: shard/transform full inputs into per-core maps."""
    import ml_dtypes
    adt = ml_dtypes.bfloat16
    x = np.ascontiguousarray(np.asarray(in_features, dtype=np.float32)[0])
    pos = np.asarray(token_positions).reshape(-1)[:seq].astype(np.float64)

    xt = np.ascontiguousarray(x.T)  # [D, S]

    # RoPE tables in rotate-half form after pair permutation, packed per
    # chunk as [cos(CH) | sin(CH)] so each chunk stages with one DMA.
    inv_freq = THETA ** (-np.arange(0, HEAD_DIM, 2, dtype=np.float64)
                         / HEAD_DIM)
    ang = pos[:, None] * inv_freq[None, :]  # [S, 32]
    cos = np.cos(ang).T.astype(np.float32)  # [32, S]
    sin = np.sin(ang).T.astype(np.float32)
    ctab = np.tile(cos, (4, 1))  # [128, S]
    stab = np.concatenate([-sin, sin, -sin, sin], axis=0)
    CH = min(512, seq)
    NCH = seq // CH
    cstab = np.empty((128, 2 * seq), dtype=np.float32)
    for sc in range(NCH):
        cstab[:, 2 * CH * sc:2 * CH * sc + CH] = ctab[:, CH * sc:CH * (sc + 1)]
        cstab[:, 2 * CH * sc + CH:2 * CH * (sc + 1)] = \
            stab[:, CH * sc:CH * (sc + 1)]

    perm = np.concatenate(
        [np.arange(0, HEAD_DIM, 2), np.arange(1, HEAD_DIM, 2)]
    )  # within-head: evens then odds

    KBC = CH // 128
    tri = np.triu(np.ones((128, 128), dtype=np.float32))
    dmask = np.ones((128, KBC * CH), dtype=np.float32)
    for j in range(KBC):
        dmask[:, CH * j:CH * j + 128 * j] = 0.0
        dmask[:, CH * j + 128 * j:CH * j + 128 * (j + 1)] = tri
    ident = np.eye(128, dtype=np.float32)
    ones = np.ones((128, max(seq // 128, 64)), dtype=np.float32)
    e01_host = np.zeros((2, 128), dtype=np.float32)
    e01_host[0, 0:64] = 1.0
    e01_host[1, 64:128] = 1.0

    WoT = np.ascontiguousarray(np.asarray(Wo, dtype=np.float32).T)  # [d, e]
    wo_packed = np.empty((128, KD * D_MODEL), dtype=np.float32)
    for e in range(KD):
        for i in range(KD):
            wo_packed[:, D_MODEL * e + 128 * i: D_MODEL * e + 128 * (i + 1)] \
                = WoT[128 * i:128 * (i + 1), 128 * e:128 * (e + 1)]

    def pack_w(Wc):
        # Wc: [128 out, 1024 in] -> WT [1024, 128] -> [128, 8*128] k-tiled
        WT = np.ascontiguousarray(Wc.T)
        return np.ascontiguousarray(
            WT.reshape(KD, 128, 128).transpose(1, 0, 2).reshape(128, KD * 128)
        ).astype(np.float32)

    in_maps = []
    for c in range(N_CORES):
        rows = slice(128 * c, 128 * (c + 1))
        Wq_r = np.asarray(Wq, dtype=np.float32)[rows].reshape(2, 64, D_MODEL)
        Wq_c = (Wq_r[:, perm, :] / math.sqrt(HEAD_DIM)).reshape(128, D_MODEL)
        Wk_r = np.asarray(Wk, dtype=np.float32)[rows].reshape(2, 64, D_MODEL)
        Wk_c = Wk_r[:, perm, :].reshape(128, D_MODEL)
        Wv_c = np.asarray(Wv, dtype=np.float32)[rows]
        in_maps.append({
            "xt": xt.astype(adt),
            "wq": pack_w(Wq_c).astype(adt),
            "wk": pack_w(Wk_c).astype(adt),
            "wv": pack_w(Wv_c).astype(adt),
            "wo": wo_packed.astype(adt),
            "cstab": cstab.astype(adt),
            "dmask": dmask.astype(adt),
            "ident": ident if DBG["f32_vt"] else ident.astype(adt),
            "ones": ones.astype(adt),
            "e01": e01_host,
        })
    return in_maps


_BUILD_CACHE = {}


def _get_nc(seq, p12_reps=1, p3_reps=1, parts="full", npass=NPASS):
    key = (seq, p12_reps, p3_reps, parts, npass)
    if key not in _BUILD_CACHE:
        _BUILD_CACHE[key] = build(seq, p12_reps, p3_reps, parts, npass)
    return _BUILD_CACHE[key]


def postprocess(results, seq, in_dtype, npass=NPASS):
    CH = min(512, seq)
    sizes, starts = _pass_spec(seq // CH, npass)
    SWPS = [CH * s // N_CORES for s in sizes]
    OFFS = [sum(SWPS[:p]) for p in range(len(sizes))]
    out = np.empty((seq, D_MODEL), dtype=np.float32)
    for c in range(N_CORES):
        res = np.asarray(results[c]["out"], dtype=np.float32)  # [D, SW]
        for p, swp in enumerate(SWPS):
            qbase = CH * starts[p] + swp * c
            out[qbase:qbase + swp, :] = res[:, OFFS[p]:OFFS[p] + swp].T
    return out.reshape(1, seq, D_MODEL).astype(in_dtype)


def kernel(in_features, token_positions, Wq, Wk, Wv, Wo):
    in_dtype = np.asarray(in_features).dtype
    B, S, D = np.asarray(in_features).shape
    assert B == 1 and D == D_MODEL

    nc = _get_nc(S)
    in_maps = prepare_in_maps(in_features, token_positions, Wq, Wk, Wv, Wo, S)
    res = run_bass_kernel_spmd(nc, in_maps, list(range(N_CORES)), trace=False)
    return postprocess(res.results, S, in_dtype)
